# revision 15
# baseline (speedup 1.0000x reference)
"""Trainium2 Bass kernel for nn_LoRALinear1d — bf16 I/O, streaming pipeline.

Math: out[b] = (W_main + a_in[b] @ a_out[b]) @ x[b] + b_main
  with a_in[b] = reshape(W_ain @ g[b], [CIN, R]),
       a_out[b] = reshape(W_aout @ g[b], [R, COUT]).

Sharding: data-parallel over batch B=8, one batch per NeuronCore.

HBM-bandwidth bound (~358 GB/s per core). bf16 x + bf16 out halves
traffic vs fp32 to ~34 MB/core => ~94 us roofline (quant err ~3e-3 vs
the 2e-2 gate). To keep the DMA engines saturated end-to-end:

 - x and out live in DRAM in chunk-major [NCH, P, t, LC] layout (host
   does the permutes), so every 2 MB chunk DMA is fully contiguous.
 - ALL x chunks are buffered in SBUF (128 KB/partition) so the load
   stream never stalls on compute.
 - W_ain is host-pre-shuffled to r-major so the LoRA fold is two rank-1
   matmuls reading the adapter rows straight out of SBUF: no gpsimd
   shuffles, no transposes anywhere, minimal preamble latency.
 - The 5 small weight loads are issued BEFORE the x flood and the
   preamble compute is emitted before the x dma_starts: HWDGE completion
   semaphores are assigned round-robin over 8 lanes in issue order and
   waits use cumulative per-lane thresholds, so anything emitted after
   the x loads ends up waiting on unrelated 2 MB x transfers.
 - Chunks run h-outer so each half-chunk store (1 MB) departs as soon
   as its 4 evictions land, keeping the store queue fed early.

Engine layout:
  Sync    - weight loads then the 8 contiguous 2 MB x loads (DMAHW lanes)
  Scalar  - half the evictions (bias via activation)
  Vector  - other half of evictions (tensor_scalar add)
  GpSimd  - out stores via SWDGE (separate DMASW semaphore lanes)
  Tensor  - adapter matvecs, rank-1 lora folds, main bf16 matmuls
"""

from contextlib import ExitStack

import ml_dtypes
import numpy as np

import concourse.bacc as bacc
import concourse.mybir as mybir
import concourse.tile as tile
from concourse.bass_utils import run_bass_kernel_spmd

B, CIN, COUT, CINFO, R, L = 8, 256, 256, 256, 2, 32768
P = 128
LC = 4096           # L elements per chunk
NCH = L // LC
CT = CIN // P       # 2 row-tiles
F32 = mybir.dt.float32
BF16 = mybir.dt.bfloat16
BF16_NP = ml_dtypes.bfloat16


def _build():
    nc = bacc.Bacc("TRN2", target_bir_lowering=False, debug=False)
    # x/out in chunk-major layout: [ci][p][t][l] with row o = t*128 + p
    x = nc.dram_tensor("x", [NCH, P, CT, LC], BF16, kind="ExternalInput").ap()
    g = nc.dram_tensor("g", [P, CINFO // P], BF16, kind="ExternalInput").ap()
    wmainT = nc.dram_tensor("wmainT", [CIN, COUT], BF16, kind="ExternalInput").ap()
    bmain = nc.dram_tensor("bmain", [COUT], F32, kind="ExternalInput").ap()
    # wainT pre-shuffled on host: [c, r*CIN + i] = W_ain[i*R + r, c]
    wainT = nc.dram_tensor("wainT", [CINFO, CIN * R], BF16, kind="ExternalInput").ap()
    # waoutT = W_aout.T: [c, r*COUT + o] (W_aout rows are already r-major)
    waoutT = nc.dram_tensor("waoutT", [CINFO, COUT * R], BF16, kind="ExternalInput").ap()
    out = nc.dram_tensor("out", [NCH, 2, P, CT, LC // 2], BF16, kind="ExternalOutput").ap()

    with tile.TileContext(nc) as tc, ExitStack() as ctx:
        consts = ctx.enter_context(tc.tile_pool(name="consts", bufs=1))
        xpool = ctx.enter_context(tc.tile_pool(name="xp", bufs=5))
        opool = ctx.enter_context(tc.tile_pool(name="op", bufs=7))

        # W_effT[i_tile][i, o] (i on partitions), bf16 for the main matmul
        weffT = [consts.tile([P, COUT], BF16, name=f"weffT{i}") for i in range(CT)]
        g_sb = consts.tile([P, CINFO // P], BF16)  # g[c] at [c%128, c//128]
        b_sb = consts.tile([P, COUT // P], F32)    # bias per o-tile column

        xts = []
        with (
            tc.tile_pool(name="pre", bufs=1) as pre,
            tc.tile_pool(name="prepsum", bufs=1, space="PSUM") as prepsum,
        ):
            # all weight loads first, at the HEAD of the sync ring: their
            # descriptors are serviced before the x flood, and the preamble
            # compute is emitted before the x dma_starts so its DMA-lane
            # semaphore thresholds don't fold in unrelated x loads (lanes
            # are assigned round-robin in issue order).
            nc.sync.dma_start(g_sb[:], g)
            wTs = {}
            for wdram, nm in ((wainT, "ain"), (waoutT, "aout")):
                wT = pre.tile([P, CINFO // P, 512], BF16, name=f"wT_{nm}", tag=f"wT_{nm}")
                nc.sync.dma_start(wT[:], wdram.rearrange("(h p) n -> p h n", p=P))
                wTs[nm] = wT
            wm = pre.tile([P, CT, COUT], BF16)
            nc.sync.dma_start(wm[:], wmainT.rearrange("(t p) o -> p t o", p=P))
            nc.sync.dma_start(b_sb[:], bmain.rearrange("(h p) -> p h", p=P))

            # adapter rows: a_flat[n] = sum_c W[n, c] g[c] = g^T @ W^T
            a_rows = {}
            for nm in ("ain", "aout"):
                a_ps = prepsum.tile([1, 512], F32, name=f"aps_{nm}", tag=f"aps_{nm}")
                for h in range(CINFO // P):
                    nc.tensor.matmul(
                        a_ps[:], g_sb[:, h:h + 1], wTs[nm][:, h, :],
                        start=(h == 0), stop=(h == CINFO // P - 1),
                    )
                a_row = pre.tile([1, 512], BF16, name=f"arow_{nm}", tag=f"arow_{nm}")
                nc.vector.tensor_copy(a_row[:], a_ps[:])
                a_rows[nm] = a_row

            # W_effT[it] = W_mainT[it] + sum_r a_in[:, r] (x) a_out[r, :]
            # (rank-1 matmuls straight off the r-major adapter rows)
            for it in range(CT):
                lora_ps = prepsum.tile([P, COUT], F32, name=f"lorap{it}", tag=f"lorap{it}")
                for r in range(R):
                    nc.tensor.matmul(
                        lora_ps[:],
                        a_rows["ain"][:, r * CIN + it * P:r * CIN + (it + 1) * P],
                        a_rows["aout"][:, r * COUT:(r + 1) * COUT],
                        start=(r == 0), stop=(r == R - 1),
                    )
                nc.vector.tensor_add(weffT[it][:], wm[:, it, :], lora_ps[:])

            # the x flood on the Sync queue, after the preamble compute in
            # program order; every chunk has its own SBUF buffer so the
            # read stream never backpressures
            for ci in range(NCH):
                x_t = xpool.tile([P, CT, LC], BF16, name="x_t")
                nc.sync.dma_start(x_t[:], x[ci])
                xts.append(x_t)

        # main loop: h-outer so both m-tiles of each half-chunk finish
        # together and the 1 MB half-store departs immediately.
        pspool = ctx.enter_context(tc.tile_pool(name="psp", bufs=4, space="PSUM"))
        EV = 1024  # eviction width: 2 PSUM banks
        HH = LC // EV // 2  # h-iterations per half chunk
        for ci in range(NCH):
            xmm = xts[ci]
            o_t = opool.tile([P, 2, CT, LC // 2], BF16, name="o_t")
            for half in range(2):
                for h in range(half * HH, (half + 1) * HH):
                    for m in range(COUT // P):
                        ps = pspool.tile([P, EV], F32, name="ps")
                        for k in range(CT):
                            for s in range(EV // 512):
                                nc.tensor.matmul(
                                    ps[:, s * 512:(s + 1) * 512],
                                    weffT[k][:, m * P:(m + 1) * P],
                                    xmm[:, k, h * EV + s * 512:h * EV + (s + 1) * 512],
                                    start=(k == 0), stop=(k == CT - 1),
                                )
                        hh = h - half * HH
                        osl = o_t[:, half, m, hh * EV:(hh + 1) * EV]
                        if (m + h) % 2 == 0:
                            nc.scalar.activation(
                                osl, ps[:],
                                mybir.ActivationFunctionType.Identity,
                                bias=b_sb[:, m:m + 1],
                            )
                        else:
                            nc.vector.tensor_scalar_add(osl, ps[:], b_sb[:, m:m + 1])
                # stores ride the SWDGE (gpsimd) path (separate DMASW lanes);
                # the [ci][half] layout makes each 1 MB store one contiguous
                # 8 KB segment per partition
                nc.gpsimd.dma_start(out[ci][half], o_t[:, half])

    nc.compile()
    return nc


_NC = None
LAST_RESULTS = None  # BassKernelResults from the most recent run


def _in_maps(x, g_out, W_main, b_main, W_ain, W_aout):
    wmainT = np.ascontiguousarray(W_main.T).astype(BF16_NP)
    bmain = np.ascontiguousarray(b_main, dtype=np.float32)
    # r-major shuffle: wainT[c, r*CIN + i] = W_ain[i*R + r, c]
    wainT = np.ascontiguousarray(
        W_ain.reshape(CIN, R, CINFO).transpose(2, 1, 0).reshape(CINFO, CIN * R)
    ).astype(BF16_NP)
    waoutT = np.ascontiguousarray(W_aout.T).astype(BF16_NP)
    maps = []
    for b in range(B):
        # chunk-major: xd[ci, p, t, l] = x[b, t*128 + p, ci*LC + l]
        xd = np.ascontiguousarray(
            x[b].reshape(CT, P, NCH, LC).transpose(2, 1, 0, 3)
        ).astype(BF16_NP)
        gd = np.ascontiguousarray(
            g_out[b, :, 0].reshape(CINFO // P, P).T
        ).astype(BF16_NP)
        maps.append({
            "x": xd,
            "g": gd,
            "wmainT": wmainT,
            "bmain": bmain,
            "wainT": wainT,
            "waoutT": waoutT,
        })
    return maps


def kernel(x, g_out, W_main, b_main, W_ain, W_aout, trace=False):
    global _NC, LAST_RESULTS
    if _NC is None:
        _NC = _build()
    maps = _in_maps(x, g_out, W_main, b_main, W_ain, W_aout)
    LAST_RESULTS = run_bass_kernel_spmd(
        _NC, maps, core_ids=list(range(B)), trace=trace
    )
    outs = []
    for b in range(B):
        od = LAST_RESULTS.results[b]["out"]  # [NCH, 2, P, CT, LC//2]
        outs.append(
            od.transpose(3, 2, 0, 1, 4).reshape(COUT, L).astype(np.float32)
        )
    return np.stack(outs, axis=0)


# revision 17
# speedup vs baseline: 1.0139x; 1.0139x over previous
"""Trainium2 Bass kernel for nn_LoRALinear1d — bf16 I/O, streaming pipeline.

Math: out[b] = (W_main + a_in[b] @ a_out[b]) @ x[b] + b_main
  with a_in[b] = reshape(W_ain @ g[b], [CIN, R]),
       a_out[b] = reshape(W_aout @ g[b], [R, COUT]).

Sharding: data-parallel over batch B=8, one batch per NeuronCore.

HBM-bandwidth bound (~358 GB/s per core). bf16 x + bf16 out halves
traffic vs fp32 to ~34 MB/core => ~94 us roofline (quant err ~3e-3 vs
the 2e-2 gate). To keep the DMA engines saturated end-to-end:

 - x and out live in DRAM in chunk-major [NCH, P, t, LC] layout (host
   does the permutes), so every 2 MB chunk DMA is fully contiguous.
 - ALL x chunks are buffered in SBUF (128 KB/partition) so the load
   stream never stalls on compute.
 - W_ain is host-pre-shuffled to r-major so the LoRA fold is two rank-1
   matmuls reading the adapter rows straight out of SBUF: no gpsimd
   shuffles, no transposes anywhere, minimal preamble latency.
 - The 5 small weight loads are issued BEFORE the x flood and the
   preamble compute is emitted before the x dma_starts: HWDGE completion
   semaphores are assigned round-robin over 8 lanes in issue order and
   waits use cumulative per-lane thresholds, so anything emitted after
   the x loads ends up waiting on unrelated 2 MB x transfers.
 - Chunks run h-outer so each half-chunk store (1 MB) departs as soon
   as its 4 evictions land, keeping the store queue fed early.

Engine layout:
  Sync    - weight loads then the 8 contiguous 2 MB x loads (DMAHW lanes)
  Scalar  - half the evictions (bias via activation)
  Vector  - other half of evictions (tensor_scalar add)
  GpSimd  - out stores via SWDGE (separate DMASW semaphore lanes)
  Tensor  - adapter matvecs, rank-1 lora folds, main bf16 matmuls
"""

from contextlib import ExitStack

import ml_dtypes
import numpy as np

import concourse.bacc as bacc
import concourse.mybir as mybir
import concourse.tile as tile
from concourse.bass_utils import run_bass_kernel_spmd

B, CIN, COUT, CINFO, R, L = 8, 256, 256, 256, 2, 32768
P = 128
LC = 4096           # L elements per chunk
NCH = L // LC
CT = CIN // P       # 2 row-tiles
F32 = mybir.dt.float32
BF16 = mybir.dt.bfloat16
BF16_NP = ml_dtypes.bfloat16


def _build():
    nc = bacc.Bacc("TRN2", target_bir_lowering=False, debug=False)
    # x/out in chunk-major layout: [ci][p][t][l] with row o = t*128 + p
    x = nc.dram_tensor("x", [NCH, P, CT, LC], BF16, kind="ExternalInput").ap()
    # g/b padded to 512 B per partition on the host: smaller per-partition
    # segments hit the SDMA read-modify-write slow path right before x0
    g = nc.dram_tensor("g", [P, 256], BF16, kind="ExternalInput").ap()
    wmainT = nc.dram_tensor("wmainT", [CIN, COUT], BF16, kind="ExternalInput").ap()
    bmain = nc.dram_tensor("bmain", [P, 128], F32, kind="ExternalInput").ap()
    # wainT pre-shuffled on host: [c, r*CIN + i] = W_ain[i*R + r, c]
    wainT = nc.dram_tensor("wainT", [CINFO, CIN * R], BF16, kind="ExternalInput").ap()
    # waoutT = W_aout.T: [c, r*COUT + o] (W_aout rows are already r-major)
    waoutT = nc.dram_tensor("waoutT", [CINFO, COUT * R], BF16, kind="ExternalInput").ap()
    out = nc.dram_tensor("out", [NCH, P, CT, LC], BF16, kind="ExternalOutput").ap()

    with tile.TileContext(nc) as tc, ExitStack() as ctx:
        consts = ctx.enter_context(tc.tile_pool(name="consts", bufs=1))
        xpool = ctx.enter_context(tc.tile_pool(name="xp", bufs=6))
        opool = ctx.enter_context(tc.tile_pool(name="op", bufs=6))

        # W_effT[i_tile][i, o] (i on partitions), bf16 for the main matmul
        weffT = [consts.tile([P, COUT], BF16, name=f"weffT{i}") for i in range(CT)]
        g_sb = consts.tile([P, 256], BF16)   # g[c] at [c%128, c//128], padded
        b_sb = consts.tile([P, 128], F32)    # bias per o-tile column, padded

        xts = []
        with (
            tc.tile_pool(name="pre", bufs=1) as pre,
            tc.tile_pool(name="prepsum", bufs=1, space="PSUM") as prepsum,
        ):
            # all weight loads first, at the HEAD of the sync ring: their
            # descriptors are serviced before the x flood, and the preamble
            # compute is emitted before the x dma_starts so its DMA-lane
            # semaphore thresholds don't fold in unrelated x loads (lanes
            # are assigned round-robin in issue order).
            nc.sync.dma_start(g_sb[:], g)
            wTs = {}
            for wdram, nm in ((wainT, "ain"), (waoutT, "aout")):
                wT = pre.tile([P, CINFO // P, 512], BF16, name=f"wT_{nm}", tag=f"wT_{nm}")
                nc.sync.dma_start(wT[:], wdram.rearrange("(h p) n -> p h n", p=P))
                wTs[nm] = wT
            wm = pre.tile([P, CT, COUT], BF16)
            nc.sync.dma_start(wm[:], wmainT.rearrange("(t p) o -> p t o", p=P))
            nc.sync.dma_start(b_sb[:], bmain)

            # adapter rows: a_flat[n] = sum_c W[n, c] g[c] = g^T @ W^T
            a_rows = {}
            for nm in ("ain", "aout"):
                a_ps = prepsum.tile([1, 512], F32, name=f"aps_{nm}", tag=f"aps_{nm}")
                for h in range(CINFO // P):
                    nc.tensor.matmul(
                        a_ps[:], g_sb[:, h:h + 1], wTs[nm][:, h, :],
                        start=(h == 0), stop=(h == CINFO // P - 1),
                    )
                a_row = pre.tile([1, 512], BF16, name=f"arow_{nm}", tag=f"arow_{nm}")
                nc.vector.tensor_copy(a_row[:], a_ps[:])
                a_rows[nm] = a_row

            # W_effT[it] = W_mainT[it] + sum_r a_in[:, r] (x) a_out[r, :]
            # (rank-1 matmuls straight off the r-major adapter rows)
            for it in range(CT):
                lora_ps = prepsum.tile([P, COUT], F32, name=f"lorap{it}", tag=f"lorap{it}")
                for r in range(R):
                    nc.tensor.matmul(
                        lora_ps[:],
                        a_rows["ain"][:, r * CIN + it * P:r * CIN + (it + 1) * P],
                        a_rows["aout"][:, r * COUT:(r + 1) * COUT],
                        start=(r == 0), stop=(r == R - 1),
                    )
                nc.vector.tensor_add(weffT[it][:], wm[:, it, :], lora_ps[:])

            # PE warm-up filler: the PE would otherwise sit idle ~10 us
            # between the preamble and chunk 0 (x0's completion semaphore),
            # which drops the HAM clock gate back to 1.2 GHz right as the
            # main loop starts. ~6 us of throwaway matmuls on already-loaded
            # weight tiles keeps the 2.4 GHz clock armed.
            warm_ps = prepsum.tile([P, 512], F32, name="warmps", tag="warmps")
            for _ in range(28):
                nc.tensor.matmul(
                    warm_ps[:], wTs["ain"][:, 0, 0:P], wTs["ain"][:, 1, :],
                    start=True, stop=True,
                )

            # the x flood on the Sync queue, after the preamble compute in
            # program order; every chunk has its own SBUF buffer so the
            # read stream never backpressures
            for ci in range(NCH):
                x_t = xpool.tile([P, CT, LC], BF16, name="x_t")
                nc.sync.dma_start(x_t[:], x[ci])
                xts.append(x_t)

        # main loop: h-outer so both m-tiles of each half-chunk finish
        # together and the 1 MB half-store departs immediately.
        pspool = ctx.enter_context(tc.tile_pool(name="psp", bufs=4, space="PSUM"))
        EV = 1024  # eviction width: 2 PSUM banks
        HH = LC // EV // 2  # h-iterations per half chunk
        for ci in range(NCH):
            xmm = xts[ci]
            o_t = opool.tile([P, CT, LC], BF16, name="o_t")
            for half in range(2):
                for h in range(half * HH, (half + 1) * HH):
                    for m in range(COUT // P):
                        ps = pspool.tile([P, EV], F32, name="ps")
                        for k in range(CT):
                            for s in range(EV // 512):
                                nc.tensor.matmul(
                                    ps[:, s * 512:(s + 1) * 512],
                                    weffT[k][:, m * P:(m + 1) * P],
                                    xmm[:, k, h * EV + s * 512:h * EV + (s + 1) * 512],
                                    start=(k == 0), stop=(k == CT - 1),
                                )
                        osl = o_t[:, m, h * EV:(h + 1) * EV]
                        if (m + h) % 2 == 0:
                            nc.scalar.activation(
                                osl, ps[:],
                                mybir.ActivationFunctionType.Identity,
                                bias=b_sb[:, m:m + 1],
                            )
                        else:
                            nc.vector.tensor_scalar_add(osl, ps[:], b_sb[:, m:m + 1])
                lo, hi = half * (LC // 2), (half + 1) * (LC // 2)
                # stores ride the SWDGE (gpsimd) path: they use the separate
                # DMASW semaphore lanes, so the 8 DMAHW lanes carry only the
                # weight+x loads and no x trigger ever waits on a store
                nc.gpsimd.dma_start(out[ci][:, :, lo:hi], o_t[:, :, lo:hi])

    nc.compile()
    return nc


_NC = None
LAST_RESULTS = None  # BassKernelResults from the most recent run


def _in_maps(x, g_out, W_main, b_main, W_ain, W_aout):
    wmainT = np.ascontiguousarray(W_main.T).astype(BF16_NP)
    bmain = np.zeros((P, 128), dtype=np.float32)
    bmain[:, :COUT // P] = b_main.reshape(COUT // P, P).T
    # r-major shuffle: wainT[c, r*CIN + i] = W_ain[i*R + r, c]
    wainT = np.ascontiguousarray(
        W_ain.reshape(CIN, R, CINFO).transpose(2, 1, 0).reshape(CINFO, CIN * R)
    ).astype(BF16_NP)
    waoutT = np.ascontiguousarray(W_aout.T).astype(BF16_NP)
    maps = []
    for b in range(B):
        # chunk-major: xd[ci, p, t, l] = x[b, t*128 + p, ci*LC + l]
        xd = np.ascontiguousarray(
            x[b].reshape(CT, P, NCH, LC).transpose(2, 1, 0, 3)
        ).astype(BF16_NP)
        gd = np.zeros((P, 256), dtype=BF16_NP)
        gd[:, :CINFO // P] = g_out[b, :, 0].reshape(CINFO // P, P).T
        maps.append({
            "x": xd,
            "g": gd,
            "wmainT": wmainT,
            "bmain": bmain,
            "wainT": wainT,
            "waoutT": waoutT,
        })
    return maps


def kernel(x, g_out, W_main, b_main, W_ain, W_aout, trace=False):
    global _NC, LAST_RESULTS
    if _NC is None:
        _NC = _build()
    maps = _in_maps(x, g_out, W_main, b_main, W_ain, W_aout)
    LAST_RESULTS = run_bass_kernel_spmd(
        _NC, maps, core_ids=list(range(B)), trace=trace
    )
    outs = []
    for b in range(B):
        od = LAST_RESULTS.results[b]["out"]  # [NCH, P, CT, LC]
        outs.append(
            od.transpose(2, 1, 0, 3).reshape(COUT, L).astype(np.float32)
        )
    return np.stack(outs, axis=0)
